# revision 18
# baseline (speedup 1.0000x reference)
"""Attention-pooling kernel for TRN2 (8 NeuronCores, batch-parallel).

Computes, for x:[32,2048,1024], W:[1024,1024], b:[1024], ctx:[1024]:
    h = tanh(x @ W + b); scores = h . ctx
    weights = softmax(scores, axis=seq)
    out = sum_s weights[s] * x[s]          -> [32, 1024]

Sharding: data-parallel over batch, 4 batches per core.

Per core: all matmuls run in float32r (full PE rate at N>=512, ~13-bit
mantissa; inputs are DMA'd with a bitcast — the PE truncates f32r
operands on read, verified bit-identical to explicit rounding).
Pass 1 computes h^T = W^T x^T per 512-row seq tile: PE transposes of x
(interleaved between matmul groups to keep the HAM clock-gate warm),
f32r matmuls accumulating h^T in PSUM, tanh+bias on ScalarE, and the
ctx-dot as an M=1 f32r matmul back on the PE producing scores [1, S].
Softmax is unnormalized (exp(s - max); the 1/Z lands on the pooled
vector). Pass 2 pools x with the transposed weight vector as the
stationary operand; it is emitted one tile late so the PE's in-order
queue never stalls on the softmax chain.
"""

import numpy as np
from contextlib import ExitStack

import concourse.bass as bass
import concourse.bacc as bacc
import concourse.mybir as mybir
import concourse.tile as tile
from concourse import masks
from concourse.bass_utils import run_bass_kernel_spmd

B, S, E, A = 32, 2048, 1024, 1024
NCORES = 8
BL = B // NCORES          # batches per core
S_TILE = 512
NCH = S_TILE // 128       # S chunks (128 rows) per tile
NT = S // S_TILE          # seq tiles per batch
KE = E // 128             # contraction chunks over embed dim
KA = A // 128             # chunks over attention dim
NC2 = S // 128            # S chunks per batch (pass 2)

F32 = mybir.dt.float32
F32R = mybir.dt.float32r
AX = mybir.AxisListType.X
AF = mybir.ActivationFunctionType


def _build(reps=1):
    nc = bacc.Bacc("TRN2", target_bir_lowering=False, debug=False,
                   num_devices=NCORES)
    x_d = nc.declare_dram_parameter("x", [BL * S, E], F32, isOutput=False)
    W_d = nc.declare_dram_parameter("W", [E, A], F32, isOutput=False)
    b_d = nc.declare_dram_parameter("b", [A], F32, isOutput=False)
    c_d = nc.declare_dram_parameter("ctx", [A], F32, isOutput=False)
    o_d = nc.declare_dram_parameter("out", [BL, E], F32, isOutput=True)

    with ExitStack() as ctx:
        tc = ctx.enter_context(tile.TileContext(nc))

        const_pool = ctx.enter_context(tc.tile_pool(name="const", bufs=1))
        xr_pool = ctx.enter_context(tc.tile_pool(name="xr", bufs=6))
        xt_pool = ctx.enter_context(tc.tile_pool(name="xT", bufs=2))
        h_pool = ctx.enter_context(tc.tile_pool(name="h", bufs=3))
        sc_pool = ctx.enter_context(tc.tile_pool(name="scores", bufs=1))
        sm_pool = ctx.enter_context(tc.tile_pool(name="softmax", bufs=1))
        out_pool = ctx.enter_context(tc.tile_pool(name="outs", bufs=1))

        ps_t = ctx.enter_context(tc.tile_pool(name="ps_t", bufs=2, space="PSUM"))
        ps_h = ctx.enter_context(tc.tile_pool(name="ps_h", bufs=2, space="PSUM"))
        ps_s = ctx.enter_context(tc.tile_pool(name="ps_s", bufs=2, space="PSUM"))
        ps_o = ctx.enter_context(tc.tile_pool(name="ps_o", bufs=1, space="PSUM"))

        # ---- constants ----
        ident = const_pool.tile([128, 128], F32)
        masks.make_identity(nc, ident[:])
        ident_r = const_pool.tile([128, 128], F32R)
        nc.vector.tensor_copy(ident_r[:], ident[:])
        ident_bf = const_pool.tile([128, 128], mybir.dt.bfloat16)
        nc.vector.tensor_copy(ident_bf[:], ident[:])

        # W -> [128, KE*A] f32r (col k*A+a holds W[k*128+p, a]); split per
        # chunk so the first matmul group only gates on chunk 0
        W_r = const_pool.tile([128, KE * A], F32R)
        for k in range(KE):
            nc.sync.dma_start(
                W_r[:, k * A:(k + 1) * A],
                W_d[k * 128:(k + 1) * 128, :].bitcast(F32R))

        b_sb = const_pool.tile([128, KA], F32)
        nc.sync.dma_start(b_sb[:], b_d.rearrange("(j p) -> p j", p=128))
        ctx_r = const_pool.tile([128, KA], F32R)
        nc.sync.dma_start(ctx_r[:],
                          c_d.bitcast(F32R).rearrange("(j p) -> p j", p=128))

        tiles = [(rep, bi, t)
                 for rep in range(reps) for bi in range(BL) for t in range(NT)]

        def dma_tile(bi, t):
            r0 = bi * S + t * S_TILE
            xr = xr_pool.tile([128, NCH * E], F32R, tag="xr")
            # per-chunk DMAs: the first transposes gate on 512KB, not 2MB
            for c in range(NCH):
                nc.sync.dma_start(
                    xr[:, c * E:(c + 1) * E],
                    x_d[r0 + c * 128: r0 + (c + 1) * 128, :].bitcast(F32R))
            return xr

        def transpose_group(xr_src, xT_dst, k):
            # 4 transposes: chunk c of S rows, contraction chunk k
            for c in range(NCH):
                tp = ps_t.tile([128, 128], F32R, tag="tps")
                nc.tensor.transpose(
                    tp[:], xr_src[:, c * E + k * 128: c * E + (k + 1) * 128],
                    ident_r[:])
                dst = xT_dst[:, k * S_TILE + c * 128: k * S_TILE + (c + 1) * 128]
                if c % 2 == 0:
                    nc.scalar.activation(dst, tp[:], AF.Copy)
                else:
                    nc.vector.tensor_copy(dst, tp[:])

        def flush_pass2(pend):
            scores_sb, batch_xrs, orow = pend
            # softmax (unnormalized)
            m_sb = sm_pool.tile([1, 1], F32, tag="m")
            nc.vector.reduce_max(m_sb[:], scores_sb[:], axis=AX)
            mneg = sm_pool.tile([1, 1], F32, tag="mneg")
            nc.scalar.activation(mneg[:], m_sb[:], AF.Copy, scale=-1.0)
            p_sb = sm_pool.tile([1, S], F32, tag="p")
            z_sb = sm_pool.tile([1, 1], F32, tag="z")
            nc.scalar.activation(p_sb[:], scores_sb[:], AF.Exp,
                                 bias=mneg[0:1, 0:1], accum_out=z_sb[:])
            rz = sm_pool.tile([1, 1], F32, tag="rz")
            nc.vector.reciprocal(rz[:], z_sb[:])

            # transpose p -> pT [128, NC2], packed 8 per PSUM bank
            pT = sm_pool.tile([128, NC2], F32R, tag="pT")
            for g in range(NC2 // 8):
                tp = ps_t.tile([128, 8], F32, tag="tps")
                for u in range(8):
                    c2 = g * 8 + u
                    nc.tensor.matmul(
                        tp[:, u:u + 1], p_sb[0:1, c2 * 128:(c2 + 1) * 128],
                        ident[0:1, 0:1],
                        is_transpose=True,
                        start=(u == 0), stop=(u == 7),
                        skip_group_check=True)
                nc.scalar.activation(pT[:, g * 8:(g + 1) * 8], tp[:], AF.Copy)

            # pass 2: pooling
            op0 = ps_o.tile([1, 512], F32, tag="op0")
            op1 = ps_o.tile([1, 512], F32, tag="op1")
            for c2 in range(NC2):
                xsrc = batch_xrs[c2 // NCH]
                cc = c2 % NCH
                nc.tensor.matmul(op0[:], pT[:, c2:c2 + 1],
                                 xsrc[:, cc * E: cc * E + 512],
                                 start=(c2 == 0), stop=(c2 == NC2 - 1))
                nc.tensor.matmul(op1[:], pT[:, c2:c2 + 1],
                                 xsrc[:, cc * E + 512: (cc + 1) * E],
                                 start=(c2 == 0), stop=(c2 == NC2 - 1))

            ob = out_pool.tile([1, E], F32, tag="ob")
            nc.vector.tensor_scalar_mul(ob[:, 0:512], op0[:], rz[0:1, 0:1])
            nc.vector.tensor_scalar_mul(ob[:, 512:1024], op1[:], rz[0:1, 0:1])
            nc.sync.dma_start(o_d[orow:orow + 1, :], ob[:])

        # prologue: first tile's data + transposes
        xr_cur = dma_tile(tiles[0][1], tiles[0][2])
        xT_cur = xt_pool.tile([128, KE * S_TILE], F32R, tag="xT")
        for k in range(KE):
            transpose_group(xr_cur, xT_cur, k)

        pending = None
        scores_sb = None
        batch_xrs = []

        for i, (rep, bi, t) in enumerate(tiles):
            if t == 0:
                scores_sb = sc_pool.tile([1, S], F32, tag="scores")
                batch_xrs = []
            batch_xrs.append(xr_cur)

            if t == 1 and pending is not None:
                flush_pass2(pending)
                pending = None

            nxt = tiles[i + 1] if i + 1 < len(tiles) else None
            if nxt is not None:
                xr_next = dma_tile(nxt[1], nxt[2])
                xT_next = xt_pool.tile([128, KE * S_TILE], F32R, tag="xT")
            else:
                xr_next = xT_next = None

            sc_ps = ps_s.tile([1, S_TILE], F32, tag="scps")
            for j in range(KA):
                hp = ps_h.tile([128, S_TILE], F32, tag="hps")
                for k in range(KE):
                    nc.tensor.matmul(
                        hp[:],
                        W_r[:, k * A + j * 128: k * A + (j + 1) * 128],
                        xT_cur[:, k * S_TILE:(k + 1) * S_TILE],
                        start=(k == 0), stop=(k == KE - 1))
                h_sb = h_pool.tile([128, S_TILE], F32R, tag="h")
                nc.scalar.activation(h_sb[:], hp[:], AF.Tanh,
                                     bias=b_sb[:, j:j + 1])
                nc.tensor.matmul(sc_ps[:], ctx_r[:, j:j + 1], h_sb[:],
                                 start=(j == 0), stop=(j == KA - 1))
                # keep the PE warm: next tile's transposes ride between
                # matmul groups
                if xT_next is not None:
                    transpose_group(xr_next, xT_next, j)
            nc.vector.tensor_copy(scores_sb[:, t * S_TILE:(t + 1) * S_TILE],
                                  sc_ps[:])

            if t == NT - 1:
                pending = (scores_sb, list(batch_xrs), bi)

            xr_cur, xT_cur = xr_next, xT_next

        if pending is not None:
            flush_pass2(pending)

    nc.compile()
    return nc


_NC_CACHE = None


def kernel(x, W, b, ctx):
    global _NC_CACHE
    if _NC_CACHE is None:
        _NC_CACHE = _build()
    nc = _NC_CACHE

    x = np.ascontiguousarray(np.asarray(x, dtype=np.float32))
    W = np.ascontiguousarray(np.asarray(W, dtype=np.float32))
    b = np.ascontiguousarray(np.asarray(b, dtype=np.float32))
    ctx = np.ascontiguousarray(np.asarray(ctx, dtype=np.float32))

    in_maps = [
        {"x": x[i * BL:(i + 1) * BL].reshape(BL * S, E), "W": W, "b": b,
         "ctx": ctx}
        for i in range(NCORES)
    ]
    res = run_bass_kernel_spmd(nc, in_maps, core_ids=list(range(NCORES)))
    return np.concatenate([res.results[i]["out"] for i in range(NCORES)],
                          axis=0)


if __name__ == "__main__":
    rng = np.random.default_rng(0)
    x = rng.standard_normal((B, S, E), dtype=np.float32)
    W = rng.standard_normal((E, A), dtype=np.float32) / np.sqrt(E)
    b = rng.standard_normal((A,), dtype=np.float32) * 0.01
    c = rng.standard_normal((A,), dtype=np.float32)
    out = kernel(x=x, W=W, b=b, ctx=c)
    print(out.shape, out.dtype)


# revision 24
# speedup vs baseline: 1.0046x; 1.0046x over previous
"""Attention-pooling kernel for TRN2 (8 NeuronCores, batch-parallel).

Computes, for x:[32,2048,1024], W:[1024,1024], b:[1024], ctx:[1024]:
    h = tanh(x @ W + b); scores = h . ctx
    weights = softmax(scores, axis=seq)
    out = sum_s weights[s] * x[s]          -> [32, 1024]

Sharding: data-parallel over batch, 4 batches per core.

Per core: all matmuls run in float32r (full PE rate at N>=512, ~13-bit
mantissa; inputs are DMA'd with a bitcast — the PE truncates f32r
operands on read, verified bit-identical to explicit rounding).
Pass 1 computes h^T = W^T x^T per 512-row seq tile: PE transposes of x
(interleaved between matmul groups to keep the HAM clock-gate warm),
f32r matmuls accumulating h^T in PSUM, tanh+bias on ScalarE, and the
ctx-dot as an M=1 f32r matmul back on the PE producing scores [1, S].
Softmax is unnormalized (exp(s - max); the 1/Z lands on the pooled
vector). Pass 2 pools x with the transposed weight vector as the
stationary operand; it is emitted one tile late so the PE's in-order
queue never stalls on the softmax chain.
"""

import numpy as np
from contextlib import ExitStack

import concourse.bass as bass
import concourse.bacc as bacc
import concourse.mybir as mybir
import concourse.tile as tile
from concourse import masks
from concourse.bass_utils import run_bass_kernel_spmd

B, S, E, A = 32, 2048, 1024, 1024
NCORES = 8
BL = B // NCORES          # batches per core
S_TILE = 512
NCH = S_TILE // 128       # S chunks (128 rows) per tile
NT = S // S_TILE          # seq tiles per batch
KE = E // 128             # contraction chunks over embed dim
KA = A // 128             # chunks over attention dim
NC2 = S // 128            # S chunks per batch (pass 2)

F32 = mybir.dt.float32
F32R = mybir.dt.float32r
AX = mybir.AxisListType.X
AF = mybir.ActivationFunctionType


def _build(reps=1):
    nc = bacc.Bacc("TRN2", target_bir_lowering=False, debug=False,
                   num_devices=NCORES)
    x_d = nc.declare_dram_parameter("x", [BL * S, E], F32, isOutput=False)
    W_d = nc.declare_dram_parameter("W", [E, A], F32, isOutput=False)
    b_d = nc.declare_dram_parameter("b", [A], F32, isOutput=False)
    c_d = nc.declare_dram_parameter("ctx", [A], F32, isOutput=False)
    o_d = nc.declare_dram_parameter("out", [BL, E], F32, isOutput=True)

    with ExitStack() as ctx:
        tc = ctx.enter_context(tile.TileContext(nc))

        const_pool = ctx.enter_context(tc.tile_pool(name="const", bufs=1))
        xr_pool = ctx.enter_context(tc.tile_pool(name="xr", bufs=6))
        xt_pool = ctx.enter_context(tc.tile_pool(name="xT", bufs=2))
        h_pool = ctx.enter_context(tc.tile_pool(name="h", bufs=3))
        sc_pool = ctx.enter_context(tc.tile_pool(name="scores", bufs=1))
        sm_pool = ctx.enter_context(tc.tile_pool(name="softmax", bufs=1))
        out_pool = ctx.enter_context(tc.tile_pool(name="outs", bufs=1))

        ps_t = ctx.enter_context(tc.tile_pool(name="ps_t", bufs=2, space="PSUM"))
        ps_h = ctx.enter_context(tc.tile_pool(name="ps_h", bufs=2, space="PSUM"))
        ps_s = ctx.enter_context(tc.tile_pool(name="ps_s", bufs=2, space="PSUM"))
        ps_o = ctx.enter_context(tc.tile_pool(name="ps_o", bufs=1, space="PSUM"))

        # ---- constants ----
        ident = const_pool.tile([128, 128], F32)
        masks.make_identity(nc, ident[:])
        ident_r = const_pool.tile([128, 128], F32R)
        nc.vector.tensor_copy(ident_r[:], ident[:])

        # W -> [128, KE*A] f32r (col k*A+a holds W[k*128+p, a]); split per
        # chunk so the first matmul group only gates on chunk 0
        W_r = const_pool.tile([128, KE * A], F32R)
        for k in range(KE):
            nc.sync.dma_start(
                W_r[:, k * A:(k + 1) * A],
                W_d[k * 128:(k + 1) * 128, :].bitcast(F32R))

        b_sb = const_pool.tile([128, KA], F32)
        nc.sync.dma_start(b_sb[:], b_d.rearrange("(j p) -> p j", p=128))
        ctx_r = const_pool.tile([128, KA], F32R)
        nc.sync.dma_start(ctx_r[:],
                          c_d.bitcast(F32R).rearrange("(j p) -> p j", p=128))

        tiles = [(rep, bi, t)
                 for rep in range(reps) for bi in range(BL) for t in range(NT)]

        def dma_tile(bi, t):
            r0 = bi * S + t * S_TILE
            xr = xr_pool.tile([128, NCH * E], F32R, tag="xr")
            # per-chunk DMAs: the first transposes gate on 512KB, not 2MB
            for c in range(NCH):
                nc.sync.dma_start(
                    xr[:, c * E:(c + 1) * E],
                    x_d[r0 + c * 128: r0 + (c + 1) * 128, :].bitcast(F32R))
            return xr

        def transpose_group(xr_src, xT_dst, k):
            # 4 transposes: chunk c of S rows, contraction chunk k
            for c in range(NCH):
                tp = ps_t.tile([128, 128], F32R, tag="tps")
                nc.tensor.transpose(
                    tp[:], xr_src[:, c * E + k * 128: c * E + (k + 1) * 128],
                    ident_r[:])
                dst = xT_dst[:, k * S_TILE + c * 128: k * S_TILE + (c + 1) * 128]
                if c % 2 == 0:
                    nc.scalar.activation(dst, tp[:], AF.Copy)
                else:
                    nc.vector.tensor_copy(dst, tp[:])

        def flush_pass2(pend):
            scores_sb, pmax_sb, batch_xrs, orow = pend
            # softmax (unnormalized); max from the per-tile partials
            m_sb = sm_pool.tile([1, 1], F32, tag="m")
            nc.vector.reduce_max(m_sb[:], pmax_sb[:], axis=AX)
            mneg = sm_pool.tile([1, 1], F32, tag="mneg")
            nc.scalar.activation(mneg[:], m_sb[:], AF.Copy, scale=-1.0)
            p_sb = sm_pool.tile([1, S], F32, tag="p")
            z_sb = sm_pool.tile([1, 1], F32, tag="z")
            nc.scalar.activation(p_sb[:], scores_sb[:], AF.Exp,
                                 bias=mneg[0:1, 0:1], accum_out=z_sb[:])
            rz = sm_pool.tile([1, 1], F32, tag="rz")
            nc.vector.reciprocal(rz[:], z_sb[:])

            # transpose p -> pT [128, NC2], packed 8 per PSUM bank
            pT = sm_pool.tile([128, NC2], F32R, tag="pT")
            for g in range(NC2 // 8):
                tp = ps_t.tile([128, 8], F32, tag="tps")
                for u in range(8):
                    c2 = g * 8 + u
                    nc.tensor.matmul(
                        tp[:, u:u + 1], p_sb[0:1, c2 * 128:(c2 + 1) * 128],
                        ident[0:1, 0:1],
                        is_transpose=True,
                        start=(u == 0), stop=(u == 7),
                        skip_group_check=True)
                nc.scalar.activation(pT[:, g * 8:(g + 1) * 8], tp[:], AF.Copy)

            # pass 2: pooling
            op0 = ps_o.tile([1, 512], F32, tag="op0")
            op1 = ps_o.tile([1, 512], F32, tag="op1")
            for c2 in range(NC2):
                xsrc = batch_xrs[c2 // NCH]
                cc = c2 % NCH
                nc.tensor.matmul(op0[:], pT[:, c2:c2 + 1],
                                 xsrc[:, cc * E: cc * E + 512],
                                 start=(c2 == 0), stop=(c2 == NC2 - 1))
                nc.tensor.matmul(op1[:], pT[:, c2:c2 + 1],
                                 xsrc[:, cc * E + 512: (cc + 1) * E],
                                 start=(c2 == 0), stop=(c2 == NC2 - 1))

            ob = out_pool.tile([1, E], F32, tag="ob")
            nc.vector.tensor_scalar_mul(ob[:, 0:512], op0[:], rz[0:1, 0:1])
            nc.vector.tensor_scalar_mul(ob[:, 512:1024], op1[:], rz[0:1, 0:1])
            nc.sync.dma_start(o_d[orow:orow + 1, :], ob[:])

        # prologue: first tile's data + transposes
        xr_cur = dma_tile(tiles[0][1], tiles[0][2])
        xT_cur = xt_pool.tile([128, KE * S_TILE], F32R, tag="xT")
        for k in range(KE):
            transpose_group(xr_cur, xT_cur, k)

        pending = None
        scores_sb = None
        batch_xrs = []

        for i, (rep, bi, t) in enumerate(tiles):
            if t == 0:
                scores_sb = sc_pool.tile([1, S], F32, tag="scores")
                pmax_sb = sc_pool.tile([1, NT], F32, tag="pmax")
                batch_xrs = []
            batch_xrs.append(xr_cur)

            if t == 1 and pending is not None:
                flush_pass2(pending)
                pending = None

            nxt = tiles[i + 1] if i + 1 < len(tiles) else None
            if nxt is not None:
                xr_next = dma_tile(nxt[1], nxt[2])
                xT_next = xt_pool.tile([128, KE * S_TILE], F32R, tag="xT")
            else:
                xr_next = xT_next = None

            sc_ps = ps_s.tile([1, S_TILE], F32, tag="scps")
            for j in range(KA):
                hp = ps_h.tile([128, S_TILE], F32, tag="hps")
                for k in range(KE):
                    nc.tensor.matmul(
                        hp[:],
                        W_r[:, k * A + j * 128: k * A + (j + 1) * 128],
                        xT_cur[:, k * S_TILE:(k + 1) * S_TILE],
                        start=(k == 0), stop=(k == KE - 1))
                h_sb = h_pool.tile([128, S_TILE], F32R, tag="h")
                nc.scalar.activation(h_sb[:], hp[:], AF.Tanh,
                                     bias=b_sb[:, j:j + 1])
                nc.tensor.matmul(sc_ps[:], ctx_r[:, j:j + 1], h_sb[:],
                                 start=(j == 0), stop=(j == KA - 1))
                # keep the PE warm: next tile's transposes ride between
                # matmul groups
                if xT_next is not None:
                    transpose_group(xr_next, xT_next, j)
            nc.vector.tensor_copy(scores_sb[:, t * S_TILE:(t + 1) * S_TILE],
                                  sc_ps[:])
            nc.vector.reduce_max(pmax_sb[:, t:t + 1], sc_ps[:], axis=AX)

            if t == NT - 1:
                pending = (scores_sb, pmax_sb, list(batch_xrs), bi)

            xr_cur, xT_cur = xr_next, xT_next

        if pending is not None:
            flush_pass2(pending)

    nc.compile()
    return nc


_NC_CACHE = None


def kernel(x, W, b, ctx):
    global _NC_CACHE
    if _NC_CACHE is None:
        _NC_CACHE = _build()
    nc = _NC_CACHE

    x = np.ascontiguousarray(np.asarray(x, dtype=np.float32))
    W = np.ascontiguousarray(np.asarray(W, dtype=np.float32))
    b = np.ascontiguousarray(np.asarray(b, dtype=np.float32))
    ctx = np.ascontiguousarray(np.asarray(ctx, dtype=np.float32))

    in_maps = [
        {"x": x[i * BL:(i + 1) * BL].reshape(BL * S, E), "W": W, "b": b,
         "ctx": ctx}
        for i in range(NCORES)
    ]
    res = run_bass_kernel_spmd(nc, in_maps, core_ids=list(range(NCORES)))
    return np.concatenate([res.results[i]["out"] for i in range(NCORES)],
                          axis=0)


if __name__ == "__main__":
    rng = np.random.default_rng(0)
    x = rng.standard_normal((B, S, E), dtype=np.float32)
    W = rng.standard_normal((E, A), dtype=np.float32) / np.sqrt(E)
    b = rng.standard_normal((A,), dtype=np.float32) * 0.01
    c = rng.standard_normal((A,), dtype=np.float32)
    out = kernel(x=x, W=W, b=b, ctx=c)
    print(out.shape, out.dtype)
